# revision 1
# baseline (speedup 1.0000x reference)
"""GCGRU (graph-conv GRU encoder/decoder) on 8 Trainium2 NeuronCores.

Sharding: data-parallel over batch B=64 -> 8 per core (per the hint);
G [3,1024,1024] and all weights replicated on every core. The T=12
encoder + 12-step decoder time loop runs sequentially on-device; the
only host traffic is the initial shard scatter and final gather.
"""
import numpy as np
import jax
import jax.numpy as jnp
from functools import partial

N = 1024   # nodes
K = 3      # cheb supports
H = 64     # hidden
C = 1      # in/out dim
T = 12     # encoder steps
HOR = 12   # decoder horizon
B = 64     # batch
M = 8      # cores

_DIN = C + H


def _gcn(G, x, W, b):
    bb, nn, cc = x.shape
    sup = jnp.einsum('kij,bjc->bikc', G, x)
    return sup.reshape(bb, nn, -1) @ W + b


def _cell(G, x_t, h, Wg, bg, Wu, bu):
    comb = jnp.concatenate([x_t, h], axis=-1)
    z, r = jnp.split(jax.nn.sigmoid(_gcn(G, comb, Wg, bg)), 2, axis=-1)
    n = jnp.tanh(_gcn(G, jnp.concatenate([x_t, r * h], axis=-1), Wu, bu))
    return z * n + (1.0 - z) * h


@partial(jax.pmap, axis_name='i',
         in_axes=(0, None, None, None, None, None, None, None, None, None, None, None))
def _run(x, G, enc_Wg, enc_bg, enc_Wu, enc_bu,
         dec_Wg, dec_bg, dec_Wu, dec_bu, proj_W, proj_b):
    bb = x.shape[0]
    h0 = jnp.zeros((bb, N, H), x.dtype)

    def enc_step(h, x_t):
        return _cell(G, x_t, h, enc_Wg, enc_bg, enc_Wu, enc_bu), None

    h, _ = jax.lax.scan(enc_step, h0, x.transpose(1, 0, 2, 3))

    y0 = jnp.zeros((bb, N, C), x.dtype)

    def dec_step(carry, _):
        h, y = carry
        h = _cell(G, y, h, dec_Wg, dec_bg, dec_Wu, dec_bu)
        out = h @ proj_W + proj_b
        return (h, out), out

    _, outs = jax.lax.scan(dec_step, (h, y0), None, length=HOR)
    return outs.transpose(1, 0, 2, 3)


def kernel(**inputs):
    x = np.asarray(inputs['x'], dtype=np.float32)
    xs = jnp.asarray(x.reshape(M, B // M, T, N, C))
    args = tuple(jnp.asarray(np.asarray(inputs[k], dtype=np.float32)) for k in
                 ('G', 'enc_Wg', 'enc_bg', 'enc_Wu', 'enc_bu',
                  'dec_Wg', 'dec_bg', 'dec_Wu', 'dec_bu', 'proj_W', 'proj_b'))
    out = _run(xs, *args)
    return np.asarray(out).reshape(B, HOR, N, C).astype(np.float32)

